# revision 45
# baseline (speedup 1.0000x reference)
"""Grouped-Query Attention (B=1, L=4096, D=1024, 16 q-heads, 4 kv-heads, hd=64)
on 8 Trainium2 NeuronCores.

Sharding: core c owns q-heads {2c, 2c+1} and their shared kv-head c//2.
Each core computes Q/K/V projections for its heads from the full (replicated)
x, runs dense softmax attention for its 2 heads, and produces a partial
output projection  attn_heads @ Wo[head_rows]  of full shape [4096, 1024].
Host sums the 8 partials and adds bo (row-parallel all-reduce on host).

v2 structure (vs v1):
  - scores for the two heads run CONCURRENTLY in 64x128 PE row-tiles
    (T0: partitions 0-63, T8: 64-127) -- K^T duplicated on both halves,
    Q^T packed h0-top/h1-bottom.  2x effective score throughput.
  - Q projection computes both heads in ONE matmul chain (M=128).
  - PV uses the [q, d] output layout: stationary = P^T chunk [k128, q128],
    moving = V|1 [k128, 65].  Full 128 output partitions; the ones-column
    yields the softmax denominator per q ON the partition axis, so the
    epilogue is a cheap per-partition reciprocal + tensor_scalar multiply.
  - av PSUM bank holds 4 q-chunk accumulation regions; a C=1 zero matmul
    opens the bank (sets has_written everywhere) so all PV matmuls
    accumulate with start=False.
  - attn [q, d] is PE-transposed back to [d, q] for the out-projection.
  - software-pipelined emission: next block's Q-proj + first score group
    are emitted before the previous block's epilogue/out-proj; K/V
    projections are emitted just-in-time inside block 0; x^T DMA is
    split L-chunk-first so the pipeline starts after ~1/8 of the load.
"""

import os

os.environ.setdefault("MYCRO_LOCAL_CACHE", "1")

import numpy as np
import ml_dtypes

import concourse.bass as bass
import concourse.bacc as bacc
import concourse.mybir as mybir
from concourse.tile import TileContext
from concourse.bass_utils import run_bass_kernel_spmd

BF16 = mybir.dt.bfloat16
F32 = mybir.dt.float32
AF = mybir.ActivationFunctionType

D = 1024
L = 4096
NHEAD = 16
NKV = 4
HD = 64
NCORES = 8
HPC = NHEAD // NCORES  # 2 q heads per core
QB = 512               # q-block width
NQB = L // QB          # 8
KT = 128               # k-tile
NKT = L // KT          # 32
KG = 3                 # k-tiles per exp group
NG = (NKT + KG - 1) // KG  # 11 groups (10x3 + 1x2)
NF = D // 128          # 8 feature chunks
SCALE = 0.125          # 1/sqrt(64)

_CACHE = {}


def _build(has_bias):
    nc = bacc.Bacc("TRN2", target_bir_lowering=False, debug=False)

    # x^T pre-permuted on host to partition-major [128, NF, L] so DMA
    # descriptors are large contiguous runs per partition
    xT = nc.declare_dram_parameter("xT", [128, NF * L], BF16, isOutput=False)
    # weights pre-permuted on host to partition-major (one big-descriptor
    # DMA each; the Sync sequencer's descriptor generation is a bottleneck)
    wq = nc.declare_dram_parameter("wq", [128, NF * HPC * HD], BF16,
                                   isOutput=False)
    wk = nc.declare_dram_parameter("wk", [128, NF * 2 * HD], BF16,
                                   isOutput=False)
    wv = nc.declare_dram_parameter("wv", [128, NF * HD], BF16, isOutput=False)
    wo = nc.declare_dram_parameter("wo", [128, D], BF16, isOutput=False)
    ident = nc.declare_dram_parameter("ident", [128, 128], BF16, isOutput=False)
    bq = nc.declare_dram_parameter("bq", [1, HPC * HD], BF16, isOutput=False)
    bk = nc.declare_dram_parameter("bk", [1, 2 * HD], BF16, isOutput=False)
    bv = nc.declare_dram_parameter("bv", [1, HD], BF16, isOutput=False)
    out = nc.declare_dram_parameter("out", [L, D], BF16, isOutput=True)

    # group boundaries: (k0, gs)
    groups = []
    k = 0
    while k < NKT:
        gs = min(KG, NKT - k)
        groups.append((k, gs))
        k += gs

    with TileContext(nc) as tc:
        with (
            tc.tile_pool(name="sing", bufs=1) as sing,
            tc.tile_pool(name="ptp", bufs=2) as ptp,
            tc.tile_pool(name="atp", bufs=2) as atp,
            tc.tile_pool(name="rsp", bufs=2) as rsp,
            tc.tile_pool(name="obp", bufs=3) as obp,
            tc.tile_pool(name="psS", bufs=2, space="PSUM") as psS,
            tc.tile_pool(name="psV", bufs=1, space="PSUM") as psV,
        ):
            # ---- resident SBUF tensors ----
            # x^T stored quarter-major: [128, quarter, f, 1024] matching the
            # DRAM layout exactly, so each partition transfers as one 16KB
            # contiguous descriptor per quarter.
            xT_sb = sing.tile([128, 4, NF, 1024], BF16)

            def xs(f, start, width):
                # slice of x^T columns [start, start+width) for f-chunk f;
                # callers never cross a 1024-column quarter boundary
                return xT_sb[:, start // 1024, f,
                             start % 1024:start % 1024 + width]
            wq_sb = sing.tile([128, NF, HPC * HD], BF16)
            wk_sb = sing.tile([128, NF, 2 * HD], BF16)
            wv_sb = sing.tile([128, NF, HD], BF16)
            wo_sb = sing.tile([128, D], BF16)   # h0 rows 0-63, h1 rows 64-127
            id_sb = sing.tile([128, 128], BF16)
            KT_sb = sing.tile([128, L], BF16)    # K^T duplicated on both halves
            QT_sb = sing.tile([128, L], BF16)    # h0 rows 0-63, h1 rows 64-127
            V_sb = sing.tile([128, NKT, HD + 1], BF16)  # col 64 = 1.0 (denom)
            zc_sb = sing.tile([1, 128], BF16)
            zr_sb = sing.tile([1, HPC * (HD + 1) * 2], BF16)  # >= 260 zeros
            if has_bias:
                bq_sb = sing.tile([1, HPC * HD], BF16)
                bk_sb = sing.tile([1, 2 * HD], BF16)
                bv_sb = sing.tile([1, HD], BF16)
                ones_b = sing.tile([1, QB], BF16)

            # x^T DMA: xT DRAM layout is [128, NF, L] (partition-major).
            # First the 512 leading L-columns (small descriptors, 16-way
            # parallel) to unblock the first projections, then the rest as
            # large contiguous per-partition descriptors.
            xT4 = xT.rearrange("p (q f l) -> p q f l", q=4, f=NF)
            nc.sync.dma_start(out=wk_sb, in_=wk.rearrange(
                "p (f c) -> p f c", f=NF))
            nc.sync.dma_start(out=wv_sb, in_=wv.rearrange(
                "p (f c) -> p f c", f=NF))
            nc.sync.dma_start(out=wq_sb, in_=wq.rearrange(
                "p (f c) -> p f c", f=NF))
            nc.sync.dma_start(out=wo_sb, in_=wo[:, :])
            nc.sync.dma_start(out=id_sb, in_=ident[:, :])
            if has_bias:
                nc.sync.dma_start(out=bq_sb, in_=bq[:, :])
                nc.sync.dma_start(out=bk_sb, in_=bk[:, :])
                nc.sync.dma_start(out=bv_sb, in_=bv[:, :])
                nc.gpsimd.memset(ones_b, 1.0)
            for f in range(NF):
                nc.sync.dma_start(out=xT_sb[:, 0, f, 0:QB],
                                  in_=xT4[:, 0, f, 0:QB])
            for f in range(NF):
                nc.sync.dma_start(out=xT_sb[:, 0, f, QB:1024],
                                  in_=xT4[:, 0, f, QB:1024])
            for qtr in range(1, 4):
                for f in range(NF):
                    nc.sync.dma_start(out=xT_sb[:, qtr, f], in_=xT4[:, qtr, f])
            nc.gpsimd.memset(V_sb[:, :, HD], 1.0)
            nc.gpsimd.memset(zc_sb, 0.0)
            nc.gpsimd.memset(zr_sb, 0.0)

            # ---- projection emitters ----
            def emit_kproj(n):
                # K^T[128, 512] block n -- wk columns host-duplicated, so one
                # M=128 chain writes K^T to both partition halves directly
                kps = psS.tile([128, QB], F32, tag="st", name="kps")
                for f in range(NF):
                    nc.tensor.matmul(kps, wk_sb[:, f, :], xs(f, QB * n, QB),
                                     start=(f == 0),
                                     stop=(not has_bias and f == NF - 1))
                if has_bias:
                    nc.tensor.matmul(kps, bk_sb, ones_b, start=False, stop=True)
                nc.vector.tensor_copy(KT_sb[:, QB * n:QB * (n + 1)], kps)

            def emit_vproj(l):
                # V[128, 64] k-tile l (natural layout, k on partitions)
                vps = psS.tile([128, HD], F32, tag="st", name="vps")
                for f in range(NF):
                    nc.tensor.matmul(vps, xs(f, KT * l, KT), wv_sb[:, f, :],
                                     start=(f == 0),
                                     stop=(not has_bias and f == NF - 1))
                if has_bias:
                    nc.tensor.matmul(vps, ones_b[:, 0:KT], bv_sb,
                                     start=False, stop=True)
                nc.vector.tensor_copy(V_sb[:, l, 0:HD], vps)

            def emit_qproj(q):
                # Q^T[128, 512] both heads in one chain (unscaled; exp scales)
                qps = psS.tile([128, QB], F32, tag="st", name="qps")
                for f in range(NF):
                    nc.tensor.matmul(qps, wq_sb[:, f, :], xs(f, QB * q, QB),
                                     start=(f == 0),
                                     stop=(not has_bias and f == NF - 1))
                if has_bias:
                    nc.tensor.matmul(qps, bq_sb, ones_b, start=False, stop=True)
                nc.vector.tensor_copy(QT_sb[:, QB * q:QB * (q + 1)], qps)

            # live tiles
            st_tiles = {}  # (q, g, h) -> score tile
            cur_pt = {}   # (h) -> current exp'd tile
            cur_av = {}   # (h) -> av accumulation tile
            cur_atn = {}  # (h) -> normalized attn [q, d]
            cur_atT = {}  # (h) -> transposed attn [d, q]

            def emit_scores(q, g):
                qs = slice(QB * q, QB * (q + 1))
                k0, gs = groups[g]
                for h in range(HPC):
                    st = psS.tile([128, KG, QB], F32, tag="st", name=f"st{h}")
                    p = 64 * h
                    for j in range(gs):
                        ks = slice(KT * (k0 + j), KT * (k0 + j + 1))
                        nc.tensor.matmul(st[:, j, :], KT_sb[p:p + HD, ks],
                                         QT_sb[p:p + HD, qs],
                                         start=True, stop=True,
                                         tile_position=(p, 0))
                    st_tiles[(q, g, h)] = st

            def emit_exp(q, g):
                k0, gs = groups[g]
                for h in range(HPC):
                    pt = ptp.tile([128, KG, QB], BF16, tag=f"pt{h}",
                                  name=f"pt{h}")
                    st = st_tiles.pop((q, g, h))
                    nc.scalar.activation(pt[:, 0:gs, :], st[:, 0:gs, :],
                                         AF.Exp, scale=SCALE)
                    cur_pt[h] = pt

            def emit_zero_av():
                for h in range(HPC):
                    av = psV.tile([128, QB // KT, HD + 1], F32, tag=f"av{h}",
                                  name=f"av{h}")
                    nc.tensor.matmul(av[:, :, :], zc_sb,
                                     zr_sb[:, 0:(QB // KT) * (HD + 1)],
                                     start=True, stop=False,
                                     skip_group_check=True)
                    cur_av[h] = av

            def emit_pv(g):
                k0, gs = groups[g]
                for h in range(HPC):
                    av = cur_av[h]
                    pt = cur_pt[h]
                    for j in range(gs):
                        last = (k0 + j == NKT - 1)
                        for qc in range(QB // KT):
                            nc.tensor.matmul(
                                av[:, qc, :],
                                pt[:, j, KT * qc:KT * (qc + 1)],
                                V_sb[:, k0 + j, :],
                                start=False, stop=last,
                                skip_group_check=True)

            def emit_epilogue_scale():
                # per-partition denom -> reciprocal -> scale.
                # h1's attn is zero-padded to [128, 128] so its PE transpose
                # lands on PSUM partitions 64-127; h0's overwrites 0-63.
                # The packed atT [128, lc, 128] then feeds C=128 single-matmul
                # out-projections against wo packed h0-top/h1-bottom.
                for h in range(HPC):
                    av = cur_av[h]
                    rsb = rsp.tile([128, QB // KT], F32, tag=f"rs{h}",
                                   name=f"rs{h}")
                    nc.vector.reciprocal(rsb, av[:, :, HD])
                    if h == 0:
                        atn = atp.tile([128, QB // KT, HD], BF16, tag="at0",
                                       name="atn0")
                        off = 0
                    else:
                        atn = atp.tile([128, QB // KT, KT], BF16, tag="at1",
                                       name="atn1")
                        nc.gpsimd.memset(atn[:, :, 0:HD], 0.0)
                        off = HD
                    for qc in range(QB // KT):
                        nc.vector.tensor_scalar_mul(
                            atn[:, qc, off:off + HD], av[:, qc, 0:HD],
                            rsb[:, qc:qc + 1])
                    cur_atn[h] = atn
                cur_atT[0] = atp.tile([128, QB // KT, KT], BF16, tag="aT",
                                      name="atT")
                cur_atT[1] = psV.tile([128, QB // KT, KT], BF16, tag="av0",
                                      name="psT")

            def emit_epilogue_transpose(qcs, chunked=False):
                psT, atT = cur_atT[1], cur_atT[0]
                for qc in qcs:
                    nc.tensor.transpose(psT[:, qc, :], cur_atn[1][:, qc, :],
                                        id_sb)
                    nc.tensor.transpose(psT[0:HD, qc, :], cur_atn[0][:, qc, :],
                                        id_sb)
                    if chunked:
                        nc.vector.tensor_copy(atT[:, qc, :], psT[:, qc, :])
                if not chunked:
                    nc.vector.tensor_copy(atT, psT)

            def emit_epilogue():
                emit_epilogue_scale()
                emit_epilogue_transpose(range(QB // KT))

            cur_ops = {}

            def emit_outproj_piece(q, lc, n, split_store=False):
                at = cur_atT[0]
                if n == 0:
                    cur_ops[lc] = psS.tile([128, 2, QB], F32, tag="st",
                                           name="ops")
                ops = cur_ops[lc]
                ns = slice(QB * n, QB * (n + 1))
                nc.tensor.matmul(ops[:, n, :], at[:, lc, :],
                                 wo_sb[:, ns], start=True, stop=True)
                if n == 1:
                    osb = obp.tile([128, D], BF16, tag="ob", name="osb")
                    r0 = QB * q + KT * lc
                    if split_store:
                        for s in range(2):
                            hs = slice(QB * s, QB * (s + 1))
                            nc.vector.tensor_copy(osb[:, hs], ops[:, s, :])
                            nc.sync.dma_start(out=out[r0:r0 + KT, hs],
                                              in_=osb[:, hs])
                    else:
                        nc.vector.tensor_copy(osb, ops)
                        nc.sync.dma_start(out=out[r0:r0 + KT, :], in_=osb)

            def emit_outproj(q, lcs, split_store=False):
                for lc in lcs:
                    for n in range(2):
                        emit_outproj_piece(q, lc, n, split_store)

            # ---- software-pipelined emission ----
            # prologue: enough K/V for group 0, Q-proj 0, scores(0, 0)
            emit_kproj(0)                      # k-tiles 0-3
            for l in range(3):
                emit_vproj(l)
            emit_qproj(0)
            emit_scores(0, 0)
            emit_kproj(1)
            for l in range(3, 6):
                emit_vproj(l)
            kdone, vdone = 2, 6

            # holds (epi_q, atT deferral) state
            for q in range(NQB):
                for g in range(NG):
                    emit_exp(q, g)
                    # JIT K/V projections during block 0
                    if q == 0 and g + 1 < NG:
                        k1, gs1 = groups[g + 1]
                        need_k = min((k1 + gs1 + 3) // 4, NQB)
                        while kdone < need_k:
                            emit_kproj(kdone)
                            kdone += 1
                        while vdone < min(k1 + gs1, NKT):
                            emit_vproj(vdone)
                            vdone += 1
                    if g + 1 < NG:
                        emit_scores(q, g + 1)
                    elif q + 1 < NQB:
                        emit_scores(q + 1, 0)
                    if g == 0:
                        if q > 0:
                            emit_epilogue()
                        emit_zero_av()
                    emit_pv(g)
                    if g == 5 and q + 1 < NQB:
                        emit_qproj(q + 1)
                    if q > 0 and 1 <= g <= 8:
                        emit_outproj_piece(q - 1, (g - 1) // 2, (g - 1) % 2)
            emit_epilogue_scale()
            for qc in range(QB // KT):
                emit_epilogue_transpose((qc,), chunked=True)
                emit_outproj(NQB - 1, (qc,), split_store=True)
    nc.finalize()
    return nc


def _prep_inputs(x, Wq, bq, Wk, bk, Wv, bv, Wo, bo):
    bf = ml_dtypes.bfloat16
    xTf = np.asarray(x, dtype=np.float32)[0].T          # [D, L]
    xT = np.ascontiguousarray(
        xTf.reshape(NF, 128, 4, 1024).transpose(1, 2, 0, 3)
        .reshape(128, NF * L)
    ).astype(bf)                  # partition-major, quarter-major per row
    Wq = np.asarray(Wq, dtype=np.float32)
    Wk = np.asarray(Wk, dtype=np.float32)
    Wv = np.asarray(Wv, dtype=np.float32)
    Wo = np.asarray(Wo, dtype=np.float32)
    bq = np.asarray(bq, dtype=np.float32)
    bk = np.asarray(bk, dtype=np.float32)
    bv = np.asarray(bv, dtype=np.float32)
    has_bias = bool(np.any(bq) or np.any(bk) or np.any(bv))
    ident = np.eye(128, dtype=np.float32).astype(bf)
    in_maps = []
    for c in range(NCORES):
        qsl = slice(HPC * HD * c, HPC * HD * (c + 1))   # this core's q-head cols
        kv = c // 2                                     # its kv head
        ksl = slice(HD * kv, HD * (kv + 1))
        def pmajor(w):
            # [D, C] -> partition-major [128, NF*C]
            c = w.shape[1]
            return np.ascontiguousarray(
                w.reshape(NF, 128, c).transpose(1, 0, 2).reshape(128, NF * c))
        wk_c = Wk[:, ksl]
        in_maps.append({
            "xT": xT,
            "wq": pmajor(Wq[:, qsl]).astype(bf),
            "wk": pmajor(np.concatenate([wk_c, wk_c], axis=1)).astype(bf),
            "wv": pmajor(Wv[:, ksl]).astype(bf),
            "wo": np.ascontiguousarray(
                Wo[HPC * HD * c:HPC * HD * (c + 1), :]).astype(bf),
            "ident": ident,
            "bq": bq[qsl].reshape(1, -1).astype(bf),
            "bk": np.concatenate([bk[ksl], bk[ksl]]).reshape(1, -1).astype(bf),
            "bv": bv[ksl].reshape(1, -1).astype(bf),
        })
    return in_maps, has_bias


def run(inputs, trace=False):
    in_maps, has_bias = _prep_inputs(**inputs)
    key = ("nc", has_bias)
    if key not in _CACHE:
        _CACHE[key] = _build(has_bias)
    nc = _CACHE[key]
    res = run_bass_kernel_spmd(nc, in_maps, list(range(NCORES)), trace=trace)
    bo = np.asarray(inputs["bo"], dtype=np.float32)
    acc = np.zeros((L, D), dtype=np.float32)
    for r in res.results:
        acc += np.asarray(r["out"], dtype=np.float32)
    out = (acc + bo).reshape(1, L, D)
    return out, res


def kernel(**inputs):
    out, _ = run(inputs, trace=False)
    return out


# revision 46
# speedup vs baseline: 1.0629x; 1.0629x over previous
"""Grouped-Query Attention (B=1, L=4096, D=1024, 16 q-heads, 4 kv-heads, hd=64)
on 8 Trainium2 NeuronCores.

Sharding: core c owns q-heads {2c, 2c+1} and their shared kv-head c//2.
Each core computes Q/K/V projections for its heads from the full (replicated)
x, runs dense softmax attention for its 2 heads, and produces a partial
output projection  attn_heads @ Wo[head_rows]  of full shape [4096, 1024].
Host sums the 8 partials and adds bo (row-parallel all-reduce on host).

v2 structure (vs v1):
  - scores for the two heads run CONCURRENTLY in 64x128 PE row-tiles
    (T0: partitions 0-63, T8: 64-127) -- K^T duplicated on both halves,
    Q^T packed h0-top/h1-bottom.  2x effective score throughput.
  - Q projection computes both heads in ONE matmul chain (M=128).
  - PV uses the [q, d] output layout: stationary = P^T chunk [k128, q128],
    moving = V|1 [k128, 65].  Full 128 output partitions; the ones-column
    yields the softmax denominator per q ON the partition axis, so the
    epilogue is a cheap per-partition reciprocal + tensor_scalar multiply.
  - av PSUM bank holds 4 q-chunk accumulation regions; a C=1 zero matmul
    opens the bank (sets has_written everywhere) so all PV matmuls
    accumulate with start=False.
  - attn [q, d] is PE-transposed back to [d, q] for the out-projection.
  - software-pipelined emission: next block's Q-proj + first score group
    are emitted before the previous block's epilogue/out-proj; K/V
    projections are emitted just-in-time inside block 0; x^T DMA is
    split L-chunk-first so the pipeline starts after ~1/8 of the load.
"""

import os

os.environ.setdefault("MYCRO_LOCAL_CACHE", "1")

import numpy as np
import ml_dtypes

import concourse.bass as bass
import concourse.bacc as bacc
import concourse.mybir as mybir
from concourse.tile import TileContext
from concourse.bass_utils import run_bass_kernel_spmd

BF16 = mybir.dt.bfloat16
F32 = mybir.dt.float32
AF = mybir.ActivationFunctionType

D = 1024
L = 4096
NHEAD = 16
NKV = 4
HD = 64
NCORES = 8
HPC = NHEAD // NCORES  # 2 q heads per core
QB = 512               # q-block width
NQB = L // QB          # 8
KT = 128               # k-tile
NKT = L // KT          # 32
KG = 3                 # k-tiles per exp group
NG = (NKT + KG - 1) // KG  # 11 groups (10x3 + 1x2)
NF = D // 128          # 8 feature chunks
SCALE = 0.125          # 1/sqrt(64)

_CACHE = {}


def _build(has_bias):
    nc = bacc.Bacc("TRN2", target_bir_lowering=False, debug=False)

    # x^T pre-permuted on host to partition-major [128, NF, L] so DMA
    # descriptors are large contiguous runs per partition
    xT = nc.declare_dram_parameter("xT", [128, NF * L], BF16, isOutput=False)
    # weights pre-permuted on host to partition-major (one big-descriptor
    # DMA each; the Sync sequencer's descriptor generation is a bottleneck)
    wq = nc.declare_dram_parameter("wq", [128, NF * HPC * HD], BF16,
                                   isOutput=False)
    wk = nc.declare_dram_parameter("wk", [128, NF * 2 * HD], BF16,
                                   isOutput=False)
    wv = nc.declare_dram_parameter("wv", [128, NF * HD], BF16, isOutput=False)
    wo = nc.declare_dram_parameter("wo", [128, D], BF16, isOutput=False)
    ident = nc.declare_dram_parameter("ident", [128, 128], BF16, isOutput=False)
    bq = nc.declare_dram_parameter("bq", [1, HPC * HD], BF16, isOutput=False)
    bk = nc.declare_dram_parameter("bk", [1, 2 * HD], BF16, isOutput=False)
    bv = nc.declare_dram_parameter("bv", [1, HD], BF16, isOutput=False)
    out = nc.declare_dram_parameter("out", [L, D], BF16, isOutput=True)

    # group boundaries: (k0, gs)
    groups = []
    k = 0
    while k < NKT:
        gs = min(KG, NKT - k)
        groups.append((k, gs))
        k += gs

    with TileContext(nc) as tc:
        with (
            tc.tile_pool(name="sing", bufs=1) as sing,
            tc.tile_pool(name="ptp", bufs=2) as ptp,
            tc.tile_pool(name="atp", bufs=2) as atp,
            tc.tile_pool(name="rsp", bufs=2) as rsp,
            tc.tile_pool(name="obp", bufs=3) as obp,
            tc.tile_pool(name="psS", bufs=2, space="PSUM") as psS,
            tc.tile_pool(name="psV", bufs=1, space="PSUM") as psV,
        ):
            # ---- resident SBUF tensors ----
            # x^T stored quarter-major: [128, quarter, f, 1024] matching the
            # DRAM layout exactly, so each partition transfers as one 16KB
            # contiguous descriptor per quarter.
            xT_sb = sing.tile([128, 4, NF, 1024], BF16)

            def xs(f, start, width):
                # slice of x^T columns [start, start+width) for f-chunk f;
                # callers never cross a 1024-column quarter boundary
                return xT_sb[:, start // 1024, f,
                             start % 1024:start % 1024 + width]
            wq_sb = sing.tile([128, NF, HPC * HD], BF16)
            wk_sb = sing.tile([128, NF, 2 * HD], BF16)
            wv_sb = sing.tile([128, NF, HD], BF16)
            wo_sb = sing.tile([128, D], BF16)   # h0 rows 0-63, h1 rows 64-127
            id_sb = sing.tile([128, 128], BF16)
            KT_sb = sing.tile([128, L], BF16)    # K^T duplicated on both halves
            QT_sb = sing.tile([128, L], BF16)    # h0 rows 0-63, h1 rows 64-127
            V_sb = sing.tile([128, NKT, HD + 1], BF16)  # col 64 = 1.0 (denom)
            zc_sb = sing.tile([1, 128], BF16)
            zr_sb = sing.tile([1, HPC * (HD + 1) * 2], BF16)  # >= 260 zeros
            if has_bias:
                bq_sb = sing.tile([1, HPC * HD], BF16)
                bk_sb = sing.tile([1, 2 * HD], BF16)
                bv_sb = sing.tile([1, HD], BF16)
                ones_b = sing.tile([1, QB], BF16)

            # x^T DMA: xT DRAM layout is [128, NF, L] (partition-major).
            # First the 512 leading L-columns (small descriptors, 16-way
            # parallel) to unblock the first projections, then the rest as
            # large contiguous per-partition descriptors.
            xT4 = xT.rearrange("p (q f l) -> p q f l", q=4, f=NF)
            nc.sync.dma_start(out=wk_sb, in_=wk.rearrange(
                "p (f c) -> p f c", f=NF))
            nc.sync.dma_start(out=wv_sb, in_=wv.rearrange(
                "p (f c) -> p f c", f=NF))
            nc.sync.dma_start(out=wq_sb, in_=wq.rearrange(
                "p (f c) -> p f c", f=NF))
            nc.sync.dma_start(out=wo_sb, in_=wo[:, :])
            nc.sync.dma_start(out=id_sb, in_=ident[:, :])
            if has_bias:
                nc.sync.dma_start(out=bq_sb, in_=bq[:, :])
                nc.sync.dma_start(out=bk_sb, in_=bk[:, :])
                nc.sync.dma_start(out=bv_sb, in_=bv[:, :])
                nc.gpsimd.memset(ones_b, 1.0)
            for f in range(NF):
                nc.sync.dma_start(out=xT_sb[:, 0, f, 0:QB],
                                  in_=xT4[:, 0, f, 0:QB])
            for f in range(NF):
                nc.sync.dma_start(out=xT_sb[:, 0, f, QB:1024],
                                  in_=xT4[:, 0, f, QB:1024])
            for qtr in range(1, 4):
                for f in range(NF):
                    nc.sync.dma_start(out=xT_sb[:, qtr, f], in_=xT4[:, qtr, f])
            nc.gpsimd.memset(V_sb[:, :, HD], 1.0)
            nc.gpsimd.memset(zc_sb, 0.0)
            nc.gpsimd.memset(zr_sb, 0.0)

            # ---- projection emitters ----
            def emit_kproj(n):
                # K^T[128, 512] block n -- wk columns host-duplicated, so one
                # M=128 chain writes K^T to both partition halves directly
                kps = psS.tile([128, QB], F32, tag="st", name="kps")
                for f in range(NF):
                    nc.tensor.matmul(kps, wk_sb[:, f, :], xs(f, QB * n, QB),
                                     start=(f == 0),
                                     stop=(not has_bias and f == NF - 1))
                if has_bias:
                    nc.tensor.matmul(kps, bk_sb, ones_b, start=False, stop=True)
                nc.vector.tensor_copy(KT_sb[:, QB * n:QB * (n + 1)], kps)

            def emit_vproj(l):
                # V[128, 64] k-tile l (natural layout, k on partitions)
                vps = psS.tile([128, HD], F32, tag="st", name="vps")
                for f in range(NF):
                    nc.tensor.matmul(vps, xs(f, KT * l, KT), wv_sb[:, f, :],
                                     start=(f == 0),
                                     stop=(not has_bias and f == NF - 1))
                if has_bias:
                    nc.tensor.matmul(vps, ones_b[:, 0:KT], bv_sb,
                                     start=False, stop=True)
                nc.vector.tensor_copy(V_sb[:, l, 0:HD], vps)

            def emit_qproj(q):
                # Q^T[128, 512] both heads in one chain (unscaled; exp scales)
                qps = psS.tile([128, QB], F32, tag="st", name="qps")
                for f in range(NF):
                    nc.tensor.matmul(qps, wq_sb[:, f, :], xs(f, QB * q, QB),
                                     start=(f == 0),
                                     stop=(not has_bias and f == NF - 1))
                if has_bias:
                    nc.tensor.matmul(qps, bq_sb, ones_b, start=False, stop=True)
                nc.vector.tensor_copy(QT_sb[:, QB * q:QB * (q + 1)], qps)

            # live tiles
            st_tiles = {}  # (q, g, h) -> score tile
            cur_pt = {}   # (h) -> current exp'd tile
            cur_av = {}   # (h) -> av accumulation tile
            cur_atn = {}  # (h) -> normalized attn [q, d]
            cur_atT = {}  # (h) -> transposed attn [d, q]

            def emit_scores(q, g):
                qs = slice(QB * q, QB * (q + 1))
                k0, gs = groups[g]
                for h in range(HPC):
                    st = psS.tile([128, KG, QB], F32, tag="st", name=f"st{h}")
                    p = 64 * h
                    for j in range(gs):
                        ks = slice(KT * (k0 + j), KT * (k0 + j + 1))
                        nc.tensor.matmul(st[:, j, :], KT_sb[p:p + HD, ks],
                                         QT_sb[p:p + HD, qs],
                                         start=True, stop=True,
                                         tile_position=(p, 0))
                    st_tiles[(q, g, h)] = st

            def emit_exp(q, g):
                k0, gs = groups[g]
                for h in range(HPC):
                    pt = ptp.tile([128, KG, QB], BF16, tag=f"pt{h}",
                                  name=f"pt{h}")
                    st = st_tiles.pop((q, g, h))
                    nc.scalar.activation(pt[:, 0:gs, :], st[:, 0:gs, :],
                                         AF.Exp, scale=SCALE)
                    cur_pt[h] = pt

            def emit_zero_av():
                for h in range(HPC):
                    av = psV.tile([128, QB // KT, HD + 1], F32, tag=f"av{h}",
                                  name=f"av{h}")
                    nc.tensor.matmul(av[:, :, :], zc_sb,
                                     zr_sb[:, 0:(QB // KT) * (HD + 1)],
                                     start=True, stop=False,
                                     skip_group_check=True)
                    cur_av[h] = av

            def emit_pv(g):
                k0, gs = groups[g]
                for h in range(HPC):
                    av = cur_av[h]
                    pt = cur_pt[h]
                    for j in range(gs):
                        last = (k0 + j == NKT - 1)
                        for qc in range(QB // KT):
                            nc.tensor.matmul(
                                av[:, qc, :],
                                pt[:, j, KT * qc:KT * (qc + 1)],
                                V_sb[:, k0 + j, :],
                                start=False, stop=last,
                                skip_group_check=True)

            def emit_epilogue_scale():
                # per-partition denom -> reciprocal -> scale.
                # h1's attn is zero-padded to [128, 128] so its PE transpose
                # lands on PSUM partitions 64-127; h0's overwrites 0-63.
                # The packed atT [128, lc, 128] then feeds C=128 single-matmul
                # out-projections against wo packed h0-top/h1-bottom.
                for h in range(HPC):
                    av = cur_av[h]
                    rsb = rsp.tile([128, QB // KT], F32, tag=f"rs{h}",
                                   name=f"rs{h}")
                    nc.vector.reciprocal(rsb, av[:, :, HD])
                    if h == 0:
                        atn = atp.tile([128, QB // KT, HD], BF16, tag="at0",
                                       name="atn0")
                        off = 0
                    else:
                        atn = atp.tile([128, QB // KT, KT], BF16, tag="at1",
                                       name="atn1")
                        nc.gpsimd.memset(atn[:, :, 0:HD], 0.0)
                        off = HD
                    for qc in range(QB // KT):
                        nc.vector.tensor_scalar_mul(
                            atn[:, qc, off:off + HD], av[:, qc, 0:HD],
                            rsb[:, qc:qc + 1])
                    cur_atn[h] = atn
                cur_atT[0] = atp.tile([128, QB // KT, KT], BF16, tag="aT",
                                      name="atT")
                cur_atT[1] = psV.tile([128, QB // KT, KT], BF16, tag="av0",
                                      name="psT")

            def emit_epilogue_transpose(qcs, chunked=False):
                psT, atT = cur_atT[1], cur_atT[0]
                for qc in qcs:
                    nc.tensor.transpose(psT[:, qc, :], cur_atn[1][:, qc, :],
                                        id_sb)
                    nc.tensor.transpose(psT[0:HD, qc, :], cur_atn[0][:, qc, :],
                                        id_sb)
                    if chunked:
                        nc.vector.tensor_copy(atT[:, qc, :], psT[:, qc, :])
                if not chunked:
                    nc.vector.tensor_copy(atT, psT)

            def emit_epilogue():
                emit_epilogue_scale()
                emit_epilogue_transpose(range(QB // KT))

            cur_ops = {}

            def emit_outproj_piece(q, lc, n, split_store=False):
                at = cur_atT[0]
                if n == 0:
                    cur_ops[lc] = psS.tile([128, 2, QB], F32, tag="st",
                                           name="ops")
                ops = cur_ops[lc]
                ns = slice(QB * n, QB * (n + 1))
                nc.tensor.matmul(ops[:, n, :], at[:, lc, :],
                                 wo_sb[:, ns], start=True, stop=True)
                if n == 1:
                    osb = obp.tile([128, D], BF16, tag="ob", name="osb")
                    r0 = QB * q + KT * lc
                    if split_store:
                        for s in range(2):
                            hs = slice(QB * s, QB * (s + 1))
                            nc.vector.tensor_copy(osb[:, hs], ops[:, s, :])
                            nc.sync.dma_start(out=out[r0:r0 + KT, hs],
                                              in_=osb[:, hs])
                    else:
                        nc.vector.tensor_copy(osb, ops)
                        nc.sync.dma_start(out=out[r0:r0 + KT, :], in_=osb)

            def emit_outproj(q, lcs, split_store=False):
                for lc in lcs:
                    for n in range(2):
                        emit_outproj_piece(q, lc, n, split_store)

            # ---- software-pipelined emission ----
            # prologue: enough K/V for group 0, Q-proj 0, scores(0, 0)
            emit_kproj(0)                      # k-tiles 0-3
            for l in range(3):
                emit_vproj(l)
            emit_qproj(0)
            emit_scores(0, 0)
            emit_kproj(1)
            for l in range(3, 6):
                emit_vproj(l)
            kdone, vdone = 2, 6

            # holds (epi_q, atT deferral) state
            for q in range(NQB):
                for g in range(NG):
                    emit_exp(q, g)
                    # JIT K/V projections during block 0
                    if q == 0 and g + 1 < NG:
                        k1, gs1 = groups[g + 1]
                        need_k = min((k1 + gs1 + 3) // 4, NQB)
                        while kdone < need_k:
                            emit_kproj(kdone)
                            kdone += 1
                        while vdone < min(k1 + gs1, NKT):
                            emit_vproj(vdone)
                            vdone += 1
                    if g + 1 < NG:
                        emit_scores(q, g + 1)
                    elif q + 1 < NQB:
                        emit_scores(q + 1, 0)
                    if g == 0:
                        if q > 0:
                            emit_epilogue()
                        emit_zero_av()
                    emit_pv(g)
                    if g == 9 and q + 1 < NQB:
                        emit_qproj(q + 1)
                    if q > 0 and 1 <= g <= 8:
                        emit_outproj_piece(q - 1, (g - 1) // 2, (g - 1) % 2)
            emit_epilogue_scale()
            for qc in range(QB // KT):
                emit_epilogue_transpose((qc,), chunked=True)
                emit_outproj(NQB - 1, (qc,), split_store=True)
    nc.finalize()
    return nc


def _prep_inputs(x, Wq, bq, Wk, bk, Wv, bv, Wo, bo):
    bf = ml_dtypes.bfloat16
    xTf = np.asarray(x, dtype=np.float32)[0].T          # [D, L]
    xT = np.ascontiguousarray(
        xTf.reshape(NF, 128, 4, 1024).transpose(1, 2, 0, 3)
        .reshape(128, NF * L)
    ).astype(bf)                  # partition-major, quarter-major per row
    Wq = np.asarray(Wq, dtype=np.float32)
    Wk = np.asarray(Wk, dtype=np.float32)
    Wv = np.asarray(Wv, dtype=np.float32)
    Wo = np.asarray(Wo, dtype=np.float32)
    bq = np.asarray(bq, dtype=np.float32)
    bk = np.asarray(bk, dtype=np.float32)
    bv = np.asarray(bv, dtype=np.float32)
    has_bias = bool(np.any(bq) or np.any(bk) or np.any(bv))
    ident = np.eye(128, dtype=np.float32).astype(bf)
    in_maps = []
    for c in range(NCORES):
        qsl = slice(HPC * HD * c, HPC * HD * (c + 1))   # this core's q-head cols
        kv = c // 2                                     # its kv head
        ksl = slice(HD * kv, HD * (kv + 1))
        def pmajor(w):
            # [D, C] -> partition-major [128, NF*C]
            c = w.shape[1]
            return np.ascontiguousarray(
                w.reshape(NF, 128, c).transpose(1, 0, 2).reshape(128, NF * c))
        wk_c = Wk[:, ksl]
        in_maps.append({
            "xT": xT,
            "wq": pmajor(Wq[:, qsl]).astype(bf),
            "wk": pmajor(np.concatenate([wk_c, wk_c], axis=1)).astype(bf),
            "wv": pmajor(Wv[:, ksl]).astype(bf),
            "wo": np.ascontiguousarray(
                Wo[HPC * HD * c:HPC * HD * (c + 1), :]).astype(bf),
            "ident": ident,
            "bq": bq[qsl].reshape(1, -1).astype(bf),
            "bk": np.concatenate([bk[ksl], bk[ksl]]).reshape(1, -1).astype(bf),
            "bv": bv[ksl].reshape(1, -1).astype(bf),
        })
    return in_maps, has_bias


def run(inputs, trace=False):
    in_maps, has_bias = _prep_inputs(**inputs)
    key = ("nc", has_bias)
    if key not in _CACHE:
        _CACHE[key] = _build(has_bias)
    nc = _CACHE[key]
    res = run_bass_kernel_spmd(nc, in_maps, list(range(NCORES)), trace=trace)
    bo = np.asarray(inputs["bo"], dtype=np.float32)
    acc = np.zeros((L, D), dtype=np.float32)
    for r in res.results:
        acc += np.asarray(r["out"], dtype=np.float32)
    out = (acc + bo).reshape(1, L, D)
    return out, res


def kernel(**inputs):
    out, _ = run(inputs, trace=False)
    return out
